# revision 25
# baseline (speedup 1.0000x reference)
"""BiLevelRoutingAttention (spiking) Trainium2 Bass kernel — v3.

Sharding: 8 cores = 4 batches x 2 L-halves. Routing (top-k over an 8x8
region-affinity matrix per batch) runs on host. Each core receives the FULL
1024 tokens of its batch (window-permuted so its 4 query windows come first)
plus a per-core routed-window index tensor; the device computes k/v spikes
once per distinct token, bounces them through a DRAM buffer and gathers the
16 routed key/value window slots with ONE indirect (index-vector) DMA on
gpsimd (SPMD: identical program on all cores, indices are data).  The
indirect gather's source AP covers the whole bounce buffer, so the tile
framework orders it after all bounce writes (the v2 per-slot register-offset
DMAs only carried a dependency on window 0 and could race the writes).

Device pipeline per core, per timestep t (LIF recurrence over t):
  x_t arrives host-scaled by beta_t = 2^(t-1) in f16; q/k via W-stationary
  single-pass f16 matmuls ([ch, tok] layout, per-partition bias in the LIF
  charge op), v via x-stationary f16 matmuls into natural [tok, ch] layout
  (bias via a rank-1 ones matmul).  LIF state u is kept in f16 (validated:
  final z-LIF stays ~20x below threshold, so f16 state error is harmless);
  charge+fire on DVE (fire gets the 4x f16 DVE mode), hard-reset on gpsimd.
  Attention per query window: S^T with 4-way row-tiled (tile_position)
  interleaved matmuls into PSUM, exp on ACT (f16, two halves), P@V and
  P@ones with 4-way column-tiled interleaved matmuls, softmax denominators
  via ones columns whose value folds in beta_t, reciprocal_approx_fast +
  multiply (f16 out) to normalize into transposed oa layout.  Proj is a
  single-pass f16 matmul in transposed layout ([ch_out, tok]) so its bias is
  per-partition; z spikes stored transposed in f16, host restores layout.
"""

import os
import sys
import numpy as np

for _p in ("/root/.axon_site/_ro/trn_rl_repo", "/opt/trn_rl_repo"):
    if os.path.isdir(_p) and _p not in sys.path:
        sys.path.append(_p)

import concourse.bass as bass
import concourse.mybir as mybir
import concourse.tile as tile
from concourse.bass import AP, IndirectOffsetOnAxis
from concourse.vector_clock import ScopedClock

# ---------------------------------------------------------------- constants
T, B, L, C = 4, 4, 1024, 256
NUM_HEADS, N_WIN, TOPK = 8, 8, 4
HD = C // NUM_HEADS            # 32
WIN = L // N_WIN               # 128
N_CORES = 8
NLOC = 4                       # query windows per core
QTOK = NLOC * WIN              # 512 query tokens per core
NTOK = 1024                    # distinct tokens per core (full batch)
NSLOT = NLOC * TOPK            # 16 gathered window slots
SEL = NSLOT * WIN              # 2048 gathered key tokens
TAU_SCALE = float(HD) ** -0.5  # attention scale
# Schraudolph f16-exp constants: bitcast_f16(int(SCH_A * S + SCH_B))
SCH_A = float(1024.0 * np.log2(np.e) * TAU_SCALE)
SCH_B = 15320.0
G = 2                          # 128-channel groups
KCH = 2                        # contraction chunks of 128
F16 = mybir.dt.float16
F32 = mybir.dt.float32
I32 = mybir.dt.int32

RESET_ENG = os.environ.get("BK_RESET", "dve")   # pool | dve (Pool lacks
# TensorScalarPtr on this target — engine check fails; keep dve)


# ------------------------------------------------------- tail-drain patch
def _patched_drain_and_barrier(self, tick_clock, wait_clock):
    nc = self.nc
    drain_inst = nc.sync.drain()
    wait_clock.add_sem_waits(
        drain_inst.ins, ScopedClock({None: tick_clock.global_clock})
    )
    waits = list(drain_inst.ins.sync_info.on_wait)
    if len(waits) > 1:
        drain_inst.ins.sync_info.on_wait = waits[:1]
        lst = nc.cur_bb.bb.instructions
        assert lst[-1] is drain_inst.ins
        lst.pop()
        for w in waits[1:]:
            nop = nc.sync.nop(nofuse=True)
            if nop.ins.sync_info is None:
                nop.ins.sync_info = mybir.SyncInfo(on_wait=[], on_update=[])
            nop.ins.sync_info.on_wait.append(w)
        lst.append(drain_inst.ins)
    nc.all_engine_barrier()
    assert self.sems is not None
    popped = nc._tile_sem_poison_stack.pop()
    assert popped is self._sem_poison
    nc.clear_and_free_semaphores(list(self.sems.allocated().values()))
    nc.all_engine_barrier()


tile.TileContext._drain_and_barrier = _patched_drain_and_barrier


# This walrus build accepts at most 1 sem-wait per instruction; move the
# excess onto same-engine NoOps inserted just before, at the BIR-JSON level.
_MAXW = 1


def _split_excess_waits(bir_bytes):
    import orjson
    d = orjson.loads(bir_bytes)
    cnt = 0
    for fn in d.get("functions", []):
        for blk in fn.get("blocks", []):
            out = []
            for ins in blk.get("instructions", []):
                si = ins.get("sync_info")
                waits = (si or {}).get("on_wait") or []
                if len(waits) > _MAXW:
                    keep = waits[:_MAXW]
                    extra = waits[_MAXW:]
                    for j, w in enumerate(extra):
                        cnt += 1
                        out.append({
                            "debug": ins.get("debug", 0),
                            "engine": ins["engine"],
                            "ins": [], "outs": [],
                            "name": f"{ins['name']}-wsplit{j}",
                            "opcode": "NoOp",
                            "sync_info": {"on_update": [], "on_wait": [w]},
                        })
                    si["on_wait"] = keep
                out.append(ins)
            blk["instructions"] = out
    return orjson.dumps(d), cnt


def _wrap_to_json(nc):
    orig = nc.to_json_bytes

    def patched():
        b, cnt = _split_excess_waits(orig())
        return b

    nc.to_json_bytes = patched
    return nc


# ------------------------------------------------------------ host helpers
def _routing_idx(x):
    """Mirror the reference's region routing; x [T,B,L,C] fp32."""
    xs = x.sum(axis=0).reshape(B, N_WIN, WIN, C)
    region = xs.sum(axis=2)                                   # [B, 8, C]
    attn_r = np.einsum("bnc,bmc->bnm", region, region) * (C ** -0.5)
    idx = np.argsort(-attn_r, axis=-1, kind="stable")[..., :TOPK]
    return idx.astype(np.int32)                               # [B, 8, 4]


# ------------------------------------------------------------- the program
def _build_program():
    nc = bass.Bass("TRN2", target_bir_lowering=False, debug=False,
                   enable_asserts=False, num_devices=N_CORES)

    def din(name, shape, dt):
        return nc.dram_tensor(name, shape, dt, kind="ExternalInput")

    # x: [T, C, NTOK] f16 (beta-scaled, window-permuted, transposed)
    xs_in = din("xs", [T, C, NTOK], F16)
    wq_in = din("wq", [C, C], F16)
    wk_in = din("wk", [C, C], F16)
    wv_in = din("wv", [C, C], F16)
    wp_in = din("wp", [C, C], F16)
    bqkp_in = din("bqkp", [T, C, 3], F32)
    bv_in = din("bv", [T, 128, C], F16)   # beta*b_v/128, replicated rows
    gidx_in = din("gidx", [128, NSLOT], I32)   # flat gather idx: p*8 + w
    zout = nc.dram_tensor("z", [T, G, 128, QTOK], F16, kind="ExternalOutput")
    kv_bufs = [nc.dram_tensor(f"kvd{i}", [128, N_WIN * 512], F16, kind="Internal")
               for i in range(4)]

    with tile.TileContext(nc) as tc:
        with (
            tc.tile_pool(name="wts", bufs=1) as wtsp,
            tc.tile_pool(name="xin", bufs=2) as xinp,
            tc.tile_pool(name="bias", bufs=3) as biasp,
            tc.tile_pool(name="state", bufs=1) as statep,
            tc.tile_pool(name="spk", bufs=2) as spkp,
            tc.tile_pool(name="sel", bufs=2) as selp,
            tc.tile_pool(name="pt", bufs=2) as ptp,
            tc.tile_pool(name="oa", bufs=2) as oap,
            tc.tile_pool(name="mm", bufs=2, space="PSUM") as mmp,
            tc.tile_pool(name="st", bufs=2, space="PSUM") as stp,
            tc.tile_pool(name="ov", bufs=2, space="PSUM") as ovp,
        ):
            # ---- persistent weights (t-invariant); spread the issue across
            # engine queues (DMA issue is ~0.5us of sequencer time each, and
            # everything serialized on sync cost ~16us of dead ramp) and
            # order so the first k-matmul's deps land first.
            wq = wtsp.tile([128, KCH, C], F16, tag="wq")
            wk = wtsp.tile([128, KCH, C], F16, tag="wk")
            wv = wtsp.tile([128, KCH, C], F16, tag="wv")
            wp = wtsp.tile([128, KCH, C], F16, tag="wp")
            for kc in range(KCH):
                nc.scalar.dma_start(wk[:, kc, :],
                                    wk_in[kc * 128:(kc + 1) * 128, :])
                nc.gpsimd.dma_start(wv[:, kc, :],
                                    wv_in[kc * 128:(kc + 1) * 128, :])
                nc.gpsimd.dma_start(wq[:, kc, :],
                                    wq_in[kc * 128:(kc + 1) * 128, :])
                nc.gpsimd.dma_start(wp[:, kc, :],
                                    wp_in[kc * 128:(kc + 1) * 128, :])

            ones_col = wtsp.tile([128, 128], F16, tag="ones_col")
            nc.vector.memset(ones_col[:, :], 1.0)

            # ---- routed gather indices (flat: p*8 + w), one indirect DMA/t
            gidx = wtsp.tile([128, NSLOT], I32, tag="gidx")
            nc.scalar.dma_start(gidx[:, :], gidx_in[:, :])

            # ---- persistent LIF state (u-form, f16), zero-init via t==0 path
            uq = statep.tile([128, G * QTOK], F16, tag="uq")
            uk = statep.tile([128, G * NTOK], F16, tag="uk")
            uv = statep.tile([128, N_WIN * C], F16, tag="uv")
            uz = statep.tile([128, G * QTOK], F16, tag="uz")

            xs_tiles, b_tiles = {}, {}

            def prefetch(tt):
                xst = xinp.tile([128, KCH, NTOK], F16, tag="xs")
                for kc in range(KCH):
                    nc.sync.dma_start(
                        xst[:, kc, :],
                        xs_in[tt, kc * 128:(kc + 1) * 128, :])
                bqkp_t = biasp.tile([128, G, 3], F32, tag="bqkp")
                nc.scalar.dma_start(
                    bqkp_t[:, :, :],
                    bqkp_in[tt].rearrange("(g p) c -> p g c", p=128))
                bvt_t = biasp.tile([128, C], F16, tag="bvt")
                nc.scalar.dma_start(bvt_t[:, :], bv_in[tt, :, :])
                xs_tiles[tt] = xst
                b_tiles[tt] = (bqkp_t, bvt_t)

            reset_eng = nc.gpsimd if RESET_ENG == "pool" else nc.vector

            def lif_step(u_ap, psum_ap, spike_ap, first, last, bias_ap, theta):
                """u += psum + b; s = u >= theta; u = u * (u < theta).

                At t==0 (u==0) the charge is psum+bias, which the ACT engine
                can do (idle at t==0: no attention yet) — frees DVE during
                the pipeline-fill phase."""
                if first:
                    if isinstance(bias_ap, float):
                        nc.scalar.activation(
                            u_ap, psum_ap, mybir.ActivationFunctionType.Copy,
                            bias=bias_ap)
                    else:
                        nc.scalar.activation(
                            u_ap, psum_ap,
                            mybir.ActivationFunctionType.Identity,
                            bias=bias_ap)
                else:
                    nc.vector.scalar_tensor_tensor(
                        u_ap, psum_ap, bias_ap, u_ap,
                        mybir.AluOpType.add, mybir.AluOpType.add)
                nc.vector.tensor_scalar(
                    spike_ap, u_ap, theta, None, mybir.AluOpType.is_ge)
                if not last:
                    reset_eng.scalar_tensor_tensor(
                        u_ap, u_ap, theta, u_ap,
                        mybir.AluOpType.is_lt, mybir.AluOpType.mult)

            def emit_qkv_gather(t, xs, bqkp, bvt):
                """Generator: qkv chunk units for timestep t; yields between
                chunks so attention(t-1) units can interleave in the
                instruction stream.  Returns (kv_sel, qs) tiles."""
                theta = float(2.0 ** t)
                kv = spkp.tile([128, N_WIN * 512], F16, tag="kv")
                kv3 = kv[:, :].rearrange("p (w x) -> p w x", x=512)
                kvd = kv_bufs[t % 4]

                # ---- k: [256ch, 1024tok], W-stationary, 2 N-chunks of 512
                for g in range(G):
                    for nch in range(2):
                        ps = mmp.tile([128, 512], F32, tag="mm")
                        for kc in range(KCH):
                            nc.tensor.matmul(
                                ps[:, :],
                                wk[:, kc, g * 128:(g + 1) * 128],
                                xs[:, kc, nch * 512:(nch + 1) * 512],
                                start=(kc == 0), stop=(kc == KCH - 1))
                        off = g * NTOK + nch * 512
                        lif_step(uk[:, off:off + 512], ps[:, :],
                                 kv3[:, 4 * nch:4 * (nch + 1),
                                     g * 128:(g + 1) * 128],
                                 t == 0, t == T - 1,
                                 bqkp[:, g, 1:2], theta)
                        yield

                # ---- v: natural [tok, ch] via x-stationary, two windows per
                # PSUM tile so the LIF runs on [128,512] chunks (half the op
                # count / fixed overheads on DVE)
                for wp_ in range(N_WIN // 2):
                    ps = mmp.tile([128, 512], F32, tag="mm")
                    for sub in range(2):
                        w = 2 * wp_ + sub
                        for kc in range(KCH):
                            nc.tensor.matmul(
                                ps[:, sub * C:(sub + 1) * C],
                                xs[:, kc, w * 128:(w + 1) * 128],
                                wv[:, kc, :],
                                start=(kc == 0), stop=False)
                        nc.tensor.matmul(
                            ps[:, sub * C:(sub + 1) * C], ones_col[:, :],
                            bvt[:, :], start=False, stop=True)
                    w0 = 2 * wp_
                    lif_step(uv[:, w0 * C:(w0 + 2) * C], ps[:, :],
                             kv3[:, w0:w0 + 2, 256:512],
                             t == 0, t == T - 1, 0.0, theta)
                    # bounce the window pair to DRAM once its spikes land
                    nc.sync.dma_start(kvd[:, wp_ * 1024:(wp_ + 1) * 1024],
                                      kv[:, wp_ * 1024:(wp_ + 1) * 1024])
                    yield

                # ---- indirect-gather the 16 routed slots in one DMA
                kv_sel = selp.tile([128, NSLOT * 512], F16, tag="kv_sel")
                nc.gpsimd.indirect_dma_start(
                    out=kv_sel[:, :],
                    out_offset=None,
                    in_=kvd[:, :].rearrange("p (w x) -> p w x", x=512),
                    in_offset=IndirectOffsetOnAxis(ap=gidx[:, :], axis=1),
                )
                yield

                # ---- q: [256ch, 512tok], W-stationary
                qs = spkp.tile([128, G * QTOK], F16, tag="qs")
                for g in range(G):
                    ps = mmp.tile([128, 512], F32, tag="mm")
                    for kc in range(KCH):
                        nc.tensor.matmul(
                            ps[:, :],
                            wq[:, kc, g * 128:(g + 1) * 128],
                            xs[:, kc, 0:QTOK],
                            start=(kc == 0), stop=(kc == KCH - 1))
                    lif_step(uq[:, g * QTOK:(g + 1) * QTOK], ps[:, :QTOK],
                             qs[:, g * QTOK:(g + 1) * QTOK], t == 0, t == T - 1,
                             bqkp[:, g, 0:1], theta)
                    yield

                _RESULT[t] = (kv_sel, qs)

            _RESULT = {}

            def emit_attention(t, kv_sel, qs, ones32, bqkp):
                """Generator: attention n-units + proj for timestep t.

                The normalize (recip+mult) of unit n is emitted at the START
                of unit n+1: by then its PV sums are long done, so the
                in-order DVE queue never parks on a stalled reciprocal and
                head-of-line-blocks the LIF ops interleaved behind it."""
                theta = float(2.0 ** t)
                oa = []
                for g in range(G):
                    oa_g = oap.tile([128, QTOK], F16, tag=f"oa{g}",
                                    name=f"oa{g}")
                    oa.append(oa_g)

                def normalize(n, ov):
                    # oa[g][:, n] = ovo * (1/ovs); ovs has beta folded in
                    # (ones32 = 1/beta) so oa = beta_t * out.
                    rs = oap.tile([128, G * 128], F32, tag="rs")
                    ov_sums = ov[:, :].rearrange(
                        "p (g x) -> p g x", x=256)[:, :, 128:256]
                    nc.vector.reciprocal_approx_fast(
                        rs[:, :].rearrange("p (g c) -> p g c", c=128),
                        ov_sums)
                    for g in range(G):
                        nc.vector.tensor_tensor(
                            oa[g][:, n * 128:(n + 1) * 128],
                            ov[:, g * 256:g * 256 + 128],
                            rs[:, g * 128:(g + 1) * 128],
                            mybir.AluOpType.mult)

                pending_norm = None
                for n in range(NLOC):
                    if pending_norm is not None:
                        normalize(*pending_norm)
                        pending_norm = None
                    ov = ovp.tile([128, 512], F32, tag="ov")
                    for g in range(G):
                        ptt = ptp.tile([128, 2048], F16, tag="ptt")
                        for half in range(2):
                            # S^T halves (heads 2*half..2*half+2), 4-way
                            # row-tiled; separate PSUM tiles (bufs=2) so the
                            # next half's matmuls overlap this half's exp.
                            stt = stp.tile([128, 1024], F32, tag="st")
                            for mp in range(NLOC):
                                s = n * TOPK + mp
                                for hh in range(2):
                                    h = 2 * half + hh
                                    nc.tensor.matmul(
                                        stt[:, hh * 512 + mp * 128:
                                            hh * 512 + (mp + 1) * 128],
                                        kv_sel[32 * h:32 * (h + 1),
                                               s * 512 + g * 128:
                                               s * 512 + (g + 1) * 128],
                                        qs[32 * h:32 * (h + 1),
                                           g * QTOK + n * 128:
                                           g * QTOK + (n + 1) * 128],
                                        start=True, stop=True,
                                        tile_position=(32 * h, 0))
                            # NOTE: a DVE Schraudolph-exp variant (writing
                            # ptt via a .bitcast(int16) AP) raced PV reads of
                            # ptt on hardware (dep tracking misses the
                            # bitcast view) — intermittent spurious z-spikes
                            # on fresh-process runs.  All exp stays on ACT.
                            nc.scalar.activation(
                                ptt[:, half * 1024:(half + 1) * 1024],
                                stt[:, :],
                                mybir.ActivationFunctionType.Exp,
                                bias=0.0, scale=TAU_SCALE)
                        # P@V and P@ones: 4-way col-tiled, h innermost
                        for mp in range(NLOC):
                            for h in range(4):
                                hg = g * 4 + h
                                s = n * TOPK + mp
                                nc.tensor.matmul(
                                    ov[32 * h:32 * (h + 1),
                                       g * 256:g * 256 + 128],
                                    kv_sel[:, s * 512 + 256 + hg * HD:
                                           s * 512 + 256 + (hg + 1) * HD],
                                    ptt[:, h * 512 + mp * 128:
                                        h * 512 + (mp + 1) * 128],
                                    start=(mp == 0), stop=(mp == 3),
                                    tile_position=(0, 32 * h))
                            for h in range(4):
                                nc.tensor.matmul(
                                    ov[32 * h:32 * (h + 1),
                                       g * 256 + 128:g * 256 + 256],
                                    ones32[:, :],
                                    ptt[:, h * 512 + mp * 128:
                                        h * 512 + (mp + 1) * 128],
                                    start=(mp == 0), stop=(mp == 3),
                                    tile_position=(0, 32 * h))
                    pending_norm = (n, ov)
                    yield

                normalize(*pending_norm)

                # ---- proj (transposed layout, f16 single pass) + LIF
                for go in range(G):
                    ps = ovp.tile([128, 512], F32, tag="ov")
                    for kc in range(KCH):
                        nc.tensor.matmul(
                            ps[:, :],
                            wp[:, kc, go * 128:(go + 1) * 128],
                            oa[kc][:, :],
                            start=(kc == 0), stop=(kc == 1))
                    zs = oap.tile([128, QTOK], F16, tag="zs")
                    lif_step(uz[:, go * QTOK:(go + 1) * QTOK], ps[:, :],
                             zs[:, :], t == 0, t == T - 1,
                             bqkp[:, go, 2:3], theta)
                    nc.gpsimd.dma_start(zout[t, go, :, :], zs[:, :])
                    yield

            prefetch(0)
            attn_gen = None
            attn_args = None
            for t in range(T):
                inv_beta = float(2.0 ** (1 - t))   # 1/beta_t
                if t + 1 < T:
                    prefetch(t + 1)
                xs = xs_tiles.pop(t)
                bqkp, bvt = b_tiles.pop(t)
                ones32 = biasp.tile([128, HD], F16, tag="ones32")
                nc.vector.memset(ones32[:, :], inv_beta)

                # interleave attention(t-1) units with qkv(t) chunks so each
                # engine's in-order queue alternates between the two streams
                qkv_gen = emit_qkv_gather(t, xs, bqkp, bvt)
                nq = 4 + 4 + 1 + 2       # k chunks + v pairs + gather + q
                na = NLOC + G if attn_gen is not None else 0
                qi = ai = 0
                while qi < nq or ai < na:
                    # pace attention units evenly through the qkv stream
                    if ai < na and (qi >= nq or ai * nq <= qi * na):
                        next(attn_gen, None)
                        ai += 1
                    else:
                        next(qkv_gen, None)
                        qi += 1
                for _ in qkv_gen:
                    pass
                kv_sel, qs = _RESULT.pop(t)
                attn_gen = emit_attention(t, kv_sel, qs, ones32, bqkp)
            for _ in attn_gen:
                pass

    # populate .instr bytes for InstISA subclasses (custom DVE ops); raw
    # Bass skips Bacc's codegen pass and walrus dies with "ISA wrong
    # length" on empty .instr otherwise.
    mybir.codegen_inst_isa_subclasses(nc)
    return _wrap_to_json(nc)


# ------------------------------------------------------------------ driver
_CACHE = {}


def kernel(x, w_qkv, b_qkv, w_proj, b_proj):
    from concourse.bass_utils import run_bass_kernel_spmd

    x = np.asarray(x, dtype=np.float32)
    w_qkv = np.asarray(w_qkv, dtype=np.float32)
    b_qkv = np.asarray(b_qkv, dtype=np.float32)
    w_proj = np.asarray(w_proj, dtype=np.float32)
    b_proj = np.asarray(b_proj, dtype=np.float32)

    idx = _routing_idx(x)
    betas = np.asarray([2.0 ** (t - 1) for t in range(T)], np.float32)

    wq_f, wk_f, wv_f = w_qkv[:, :C], w_qkv[:, C:2 * C], w_qkv[:, 2 * C:]
    bqv, bkv, bvv = b_qkv[:C], b_qkv[C:2 * C], b_qkv[2 * C:]

    shared = dict(
        wq=wq_f.astype(np.float16), wk=wk_f.astype(np.float16),
        wv=wv_f.astype(np.float16), wp=w_proj.astype(np.float16),
        bqkp=np.stack([betas[:, None] * bqv[None, :],
                       betas[:, None] * bkv[None, :],
                       betas[:, None] * b_proj[None, :]],
                      axis=2).astype(np.float32),
        bv=np.broadcast_to(
            (betas[:, None] * bvv[None, :] / 128.0).astype(np.float16)[:, None, :],
            (T, 128, C)).copy(),
    )

    in_maps = []
    for core in range(N_CORES):
        b, half = core // 2, core % 2
        perm = list(range(half * 4, half * 4 + 4)) + \
            [w for w in range(N_WIN) if not (half * 4 <= w < half * 4 + 4)]
        pos_of = {w: i for i, w in enumerate(perm)}
        xb = x[:, b]                                         # [T, 1024, C]
        xperm = np.concatenate([xb[:, w * WIN:(w + 1) * WIN] for w in perm],
                               axis=1)                       # [T, 1024, C]
        xsT = np.swapaxes(xperm, 1, 2)                       # [T, C, 1024]
        xs_arr = np.ascontiguousarray(
            xsT * betas[:, None, None]).astype(np.float16)
        wsel = np.zeros(NSLOT, np.int64)
        for n in range(NLOC):
            for j in range(TOPK):
                wsel[n * TOPK + j] = pos_of[int(idx[b, half * 4 + n, j])]
        gidx = (np.arange(128, dtype=np.int64)[:, None] * N_WIN +
                wsel[None, :]).astype(np.int32)              # [128, 16]
        m = dict(shared)
        m.update(xs=xs_arr, gidx=gidx)
        in_maps.append(m)

    key = "prog"
    if key not in _CACHE:
        _CACHE[key] = _build_program()
    nc = _CACHE[key]

    trace = bool(int(os.environ.get("BK_TRACE", "0")))
    res = run_bass_kernel_spmd(
        nc, in_maps, core_ids=list(range(N_CORES)), trace=trace)
    if trace and res.exec_time_ns:
        print(f"HW exec time: {res.exec_time_ns} ns")

    out = np.zeros((T, B, L, C), np.float32)
    for core in range(N_CORES):
        b, half = core // 2, core % 2
        z = res.results[core]["z"]                           # [T, G, 128, 512]
        zt = np.concatenate([z[:, 0], z[:, 1]], axis=1)      # [T, 256, 512]
        out[:, b, half * QTOK:(half + 1) * QTOK, :] = \
            np.swapaxes(zt, 1, 2).astype(np.float32)
    return out


# revision 26
# speedup vs baseline: 1.0301x; 1.0301x over previous
"""BiLevelRoutingAttention (spiking) Trainium2 Bass kernel — v3.

Sharding: 8 cores = 4 batches x 2 L-halves. Routing (top-k over an 8x8
region-affinity matrix per batch) runs on host. Each core receives the FULL
1024 tokens of its batch (window-permuted so its 4 query windows come first)
plus a per-core routed-window index tensor; the device computes k/v spikes
once per distinct token, bounces them through a DRAM buffer and gathers the
16 routed key/value window slots with ONE indirect (index-vector) DMA on
gpsimd (SPMD: identical program on all cores, indices are data).  The
indirect gather's source AP covers the whole bounce buffer, so the tile
framework orders it after all bounce writes (the v2 per-slot register-offset
DMAs only carried a dependency on window 0 and could race the writes).

Device pipeline per core, per timestep t (LIF recurrence over t):
  x_t arrives host-scaled by beta_t = 2^(t-1) in f16; q/k via W-stationary
  single-pass f16 matmuls ([ch, tok] layout, per-partition bias in the LIF
  charge op), v via x-stationary f16 matmuls into natural [tok, ch] layout
  (bias via a rank-1 ones matmul).  LIF state u is kept in f16 (validated:
  final z-LIF stays ~20x below threshold, so f16 state error is harmless);
  charge+fire+reset on DVE (fire gets the 4x f16 DVE mode).
  Attention per query window: S^T with 4-way row-tiled (tile_position)
  interleaved matmuls into PSUM, exp on ACT (f16, two halves), P@V and
  P@ones with 4-way column-tiled interleaved matmuls, softmax denominators
  via ones columns whose value folds in beta_t, reciprocal_approx_fast +
  multiply (f16 out) to normalize into transposed oa layout.  Proj is a
  single-pass f16 matmul in transposed layout ([ch_out, tok]) so its bias is
  per-partition; z spikes stored transposed in f16, host restores layout.
"""

import os
import sys
import numpy as np

for _p in ("/root/.axon_site/_ro/trn_rl_repo", "/opt/trn_rl_repo"):
    if os.path.isdir(_p) and _p not in sys.path:
        sys.path.append(_p)

import concourse.bass as bass
import concourse.mybir as mybir
import concourse.tile as tile
from concourse.bass import AP, IndirectOffsetOnAxis
from concourse.vector_clock import ScopedClock

# ---------------------------------------------------------------- constants
T, B, L, C = 4, 4, 1024, 256
NUM_HEADS, N_WIN, TOPK = 8, 8, 4
HD = C // NUM_HEADS            # 32
WIN = L // N_WIN               # 128
N_CORES = 8
NLOC = 4                       # query windows per core
QTOK = NLOC * WIN              # 512 query tokens per core
NTOK = 1024                    # distinct tokens per core (full batch)
NSLOT = NLOC * TOPK            # 16 gathered window slots
SEL = NSLOT * WIN              # 2048 gathered key tokens
TAU_SCALE = float(HD) ** -0.5  # attention scale
# Schraudolph f16-exp constants: bitcast_f16(int(SCH_A * S + SCH_B))
SCH_A = float(1024.0 * np.log2(np.e) * TAU_SCALE)
SCH_B = 15320.0
G = 2                          # 128-channel groups
KCH = 2                        # contraction chunks of 128
F16 = mybir.dt.float16
F32 = mybir.dt.float32
I32 = mybir.dt.int32

RESET_ENG = os.environ.get("BK_RESET", "dve")   # pool | dve (Pool lacks
# TensorScalarPtr on this target — engine check fails; keep dve)


# ------------------------------------------------------- tail-drain patch
def _patched_drain_and_barrier(self, tick_clock, wait_clock):
    nc = self.nc
    drain_inst = nc.sync.drain()
    wait_clock.add_sem_waits(
        drain_inst.ins, ScopedClock({None: tick_clock.global_clock})
    )
    waits = list(drain_inst.ins.sync_info.on_wait)
    if len(waits) > 1:
        drain_inst.ins.sync_info.on_wait = waits[:1]
        lst = nc.cur_bb.bb.instructions
        assert lst[-1] is drain_inst.ins
        lst.pop()
        for w in waits[1:]:
            nop = nc.sync.nop(nofuse=True)
            if nop.ins.sync_info is None:
                nop.ins.sync_info = mybir.SyncInfo(on_wait=[], on_update=[])
            nop.ins.sync_info.on_wait.append(w)
        lst.append(drain_inst.ins)
    nc.all_engine_barrier()
    assert self.sems is not None
    popped = nc._tile_sem_poison_stack.pop()
    assert popped is self._sem_poison
    nc.clear_and_free_semaphores(list(self.sems.allocated().values()))
    nc.all_engine_barrier()


tile.TileContext._drain_and_barrier = _patched_drain_and_barrier


# This walrus build accepts at most 1 sem-wait per instruction; move the
# excess onto same-engine NoOps inserted just before, at the BIR-JSON level.
_MAXW = 1


def _split_excess_waits(bir_bytes):
    import orjson
    d = orjson.loads(bir_bytes)
    cnt = 0
    for fn in d.get("functions", []):
        for blk in fn.get("blocks", []):
            out = []
            for ins in blk.get("instructions", []):
                si = ins.get("sync_info")
                waits = (si or {}).get("on_wait") or []
                if len(waits) > _MAXW:
                    keep = waits[:_MAXW]
                    extra = waits[_MAXW:]
                    for j, w in enumerate(extra):
                        cnt += 1
                        out.append({
                            "debug": ins.get("debug", 0),
                            "engine": ins["engine"],
                            "ins": [], "outs": [],
                            "name": f"{ins['name']}-wsplit{j}",
                            "opcode": "NoOp",
                            "sync_info": {"on_update": [], "on_wait": [w]},
                        })
                    si["on_wait"] = keep
                out.append(ins)
            blk["instructions"] = out
    return orjson.dumps(d), cnt


def _wrap_to_json(nc):
    orig = nc.to_json_bytes

    def patched():
        b, cnt = _split_excess_waits(orig())
        return b

    nc.to_json_bytes = patched
    return nc


# ------------------------------------------------------------ host helpers
def _routing_idx(x):
    """Mirror the reference's region routing; x [T,B,L,C] fp32."""
    xs = x.sum(axis=0).reshape(B, N_WIN, WIN, C)
    region = xs.sum(axis=2)                                   # [B, 8, C]
    attn_r = np.einsum("bnc,bmc->bnm", region, region) * (C ** -0.5)
    idx = np.argsort(-attn_r, axis=-1, kind="stable")[..., :TOPK]
    return idx.astype(np.int32)                               # [B, 8, 4]


# ------------------------------------------------------------- the program
def _build_program():
    nc = bass.Bass("TRN2", target_bir_lowering=False, debug=False,
                   enable_asserts=False, num_devices=N_CORES)

    def din(name, shape, dt):
        return nc.dram_tensor(name, shape, dt, kind="ExternalInput")

    # x: [T, C, NTOK] f16 (beta-scaled, window-permuted, transposed)
    xs_in = din("xs", [T, C, NTOK], F16)
    wq_in = din("wq", [C, C], F16)
    wk_in = din("wk", [C, C], F16)
    wv_in = din("wv", [C, C], F16)
    wp_in = din("wp", [C, C], F16)
    bqkp_in = din("bqkp", [T, C, 3], F32)
    bv_in = din("bv", [T, 128, C], F16)   # beta*b_v/128, replicated rows
    gidx_in = din("gidx", [128, NSLOT], I32)   # flat gather idx: p*8 + w
    zout = nc.dram_tensor("z", [T, G, 128, QTOK], F16, kind="ExternalOutput")
    kv_bufs = [nc.dram_tensor(f"kvd{i}", [128, N_WIN * 512], F16, kind="Internal")
               for i in range(4)]

    with tile.TileContext(nc) as tc:
        with (
            tc.tile_pool(name="wts", bufs=1) as wtsp,
            tc.tile_pool(name="xin", bufs=2) as xinp,
            tc.tile_pool(name="bias", bufs=3) as biasp,
            tc.tile_pool(name="state", bufs=1) as statep,
            tc.tile_pool(name="spk", bufs=2) as spkp,
            tc.tile_pool(name="sel", bufs=2) as selp,
            tc.tile_pool(name="pt", bufs=2) as ptp,
            tc.tile_pool(name="oa", bufs=2) as oap,
            tc.tile_pool(name="mm", bufs=2, space="PSUM") as mmp,
            tc.tile_pool(name="st", bufs=2, space="PSUM") as stp,
            tc.tile_pool(name="ov", bufs=2, space="PSUM") as ovp,
        ):
            # ---- persistent weights (t-invariant); spread the issue across
            # engine queues (DMA issue is ~0.5us of sequencer time each, and
            # everything serialized on sync cost ~16us of dead ramp) and
            # order so the first k-matmul's deps land first.
            wq = wtsp.tile([128, KCH, C], F16, tag="wq")
            wk = wtsp.tile([128, KCH, C], F16, tag="wk")
            wv = wtsp.tile([128, KCH, C], F16, tag="wv")
            wp = wtsp.tile([128, KCH, C], F16, tag="wp")
            for kc in range(KCH):
                nc.scalar.dma_start(wk[:, kc, :],
                                    wk_in[kc * 128:(kc + 1) * 128, :])
                nc.gpsimd.dma_start(wv[:, kc, :],
                                    wv_in[kc * 128:(kc + 1) * 128, :])
                nc.gpsimd.dma_start(wq[:, kc, :],
                                    wq_in[kc * 128:(kc + 1) * 128, :])
                nc.gpsimd.dma_start(wp[:, kc, :],
                                    wp_in[kc * 128:(kc + 1) * 128, :])

            ones_col = wtsp.tile([128, 128], F16, tag="ones_col")
            nc.vector.memset(ones_col[:, :], 1.0)

            # ---- routed gather indices (flat: p*8 + w), one indirect DMA/t
            gidx = wtsp.tile([128, NSLOT], I32, tag="gidx")
            nc.scalar.dma_start(gidx[:, :], gidx_in[:, :])

            # ---- persistent LIF state (u-form, f16), zero-init via t==0 path
            uq = statep.tile([128, G * QTOK], F16, tag="uq")
            uk = statep.tile([128, G * NTOK], F16, tag="uk")
            uv = statep.tile([128, N_WIN * C], F16, tag="uv")
            uz = statep.tile([128, G * QTOK], F16, tag="uz")

            xs_tiles, b_tiles = {}, {}

            def prefetch(tt):
                xst = xinp.tile([128, KCH, NTOK], F16, tag="xs")
                for kc in range(KCH):
                    nc.sync.dma_start(
                        xst[:, kc, :],
                        xs_in[tt, kc * 128:(kc + 1) * 128, :])
                bqkp_t = biasp.tile([128, G, 3], F32, tag="bqkp")
                nc.scalar.dma_start(
                    bqkp_t[:, :, :],
                    bqkp_in[tt].rearrange("(g p) c -> p g c", p=128))
                bvt_t = biasp.tile([128, C], F16, tag="bvt")
                nc.scalar.dma_start(bvt_t[:, :], bv_in[tt, :, :])
                xs_tiles[tt] = xst
                b_tiles[tt] = (bqkp_t, bvt_t)

            reset_eng = nc.gpsimd if RESET_ENG == "pool" else nc.vector

            def lif_step(u_ap, psum_ap, spike_ap, first, last, bias_ap, theta):
                """u += psum + b; s = u >= theta; u = u * (u < theta).

                At t==0 (u==0) the charge is psum+bias, which the ACT engine
                can do (idle at t==0: no attention yet) — frees DVE during
                the pipeline-fill phase."""
                if first:
                    if isinstance(bias_ap, float):
                        nc.scalar.activation(
                            u_ap, psum_ap, mybir.ActivationFunctionType.Copy,
                            bias=bias_ap)
                    else:
                        nc.scalar.activation(
                            u_ap, psum_ap,
                            mybir.ActivationFunctionType.Identity,
                            bias=bias_ap)
                else:
                    nc.vector.scalar_tensor_tensor(
                        u_ap, psum_ap, bias_ap, u_ap,
                        mybir.AluOpType.add, mybir.AluOpType.add)
                nc.vector.tensor_scalar(
                    spike_ap, u_ap, theta, None, mybir.AluOpType.is_ge)
                if not last:
                    reset_eng.scalar_tensor_tensor(
                        u_ap, u_ap, theta, u_ap,
                        mybir.AluOpType.is_lt, mybir.AluOpType.mult)

            def emit_qkv_gather(t, xs, bqkp, bvt):
                """Generator: qkv chunk units for timestep t; yields between
                chunks so attention(t-1) units can interleave in the
                instruction stream.  Returns (kv_sel, qs) tiles."""
                theta = float(2.0 ** t)
                kv = spkp.tile([128, N_WIN * 512], F16, tag="kv")
                kv3 = kv[:, :].rearrange("p (w x) -> p w x", x=512)
                kvd = kv_bufs[t % 4]

                # ---- k: [256ch, 1024tok], W-stationary, 2 N-chunks of 512
                for g in range(G):
                    for nch in range(2):
                        ps = mmp.tile([128, 512], F32, tag="mm")
                        for kc in range(KCH):
                            nc.tensor.matmul(
                                ps[:, :],
                                wk[:, kc, g * 128:(g + 1) * 128],
                                xs[:, kc, nch * 512:(nch + 1) * 512],
                                start=(kc == 0), stop=(kc == KCH - 1))
                        off = g * NTOK + nch * 512
                        lif_step(uk[:, off:off + 512], ps[:, :],
                                 kv3[:, 4 * nch:4 * (nch + 1),
                                     g * 128:(g + 1) * 128],
                                 t == 0, t == T - 1,
                                 bqkp[:, g, 1:2], theta)
                        yield

                # ---- v: natural [tok, ch] via x-stationary, two windows per
                # PSUM tile so the LIF runs on [128,512] chunks (half the op
                # count / fixed overheads on DVE)
                for wp_ in range(N_WIN // 2):
                    ps = mmp.tile([128, 512], F32, tag="mm")
                    for sub in range(2):
                        w = 2 * wp_ + sub
                        for kc in range(KCH):
                            nc.tensor.matmul(
                                ps[:, sub * C:(sub + 1) * C],
                                xs[:, kc, w * 128:(w + 1) * 128],
                                wv[:, kc, :],
                                start=(kc == 0), stop=False)
                        nc.tensor.matmul(
                            ps[:, sub * C:(sub + 1) * C], ones_col[:, :],
                            bvt[:, :], start=False, stop=True)
                    w0 = 2 * wp_
                    lif_step(uv[:, w0 * C:(w0 + 2) * C], ps[:, :],
                             kv3[:, w0:w0 + 2, 256:512],
                             t == 0, t == T - 1, 0.0, theta)
                    # bounce the window pair to DRAM once its spikes land
                    nc.sync.dma_start(kvd[:, wp_ * 1024:(wp_ + 1) * 1024],
                                      kv[:, wp_ * 1024:(wp_ + 1) * 1024])
                    yield

                # ---- indirect-gather the 16 routed slots in one DMA
                kv_sel = selp.tile([128, NSLOT * 512], F16, tag="kv_sel")
                nc.gpsimd.indirect_dma_start(
                    out=kv_sel[:, :],
                    out_offset=None,
                    in_=kvd[:, :].rearrange("p (w x) -> p w x", x=512),
                    in_offset=IndirectOffsetOnAxis(ap=gidx[:, :], axis=1),
                )
                yield

                # ---- q: [256ch, 512tok], W-stationary
                qs = spkp.tile([128, G * QTOK], F16, tag="qs")
                for g in range(G):
                    ps = mmp.tile([128, 512], F32, tag="mm")
                    for kc in range(KCH):
                        nc.tensor.matmul(
                            ps[:, :],
                            wq[:, kc, g * 128:(g + 1) * 128],
                            xs[:, kc, 0:QTOK],
                            start=(kc == 0), stop=(kc == KCH - 1))
                    lif_step(uq[:, g * QTOK:(g + 1) * QTOK], ps[:, :QTOK],
                             qs[:, g * QTOK:(g + 1) * QTOK], t == 0, t == T - 1,
                             bqkp[:, g, 0:1], theta)
                    yield

                _RESULT[t] = (kv_sel, qs)

            _RESULT = {}

            def emit_attention(t, kv_sel, qs, ones32, bqkp):
                """Generator: attention n-units + proj for timestep t.

                The normalize (recip+mult) of unit n is emitted at the START
                of unit n+1: by then its PV sums are long done, so the
                in-order DVE queue never parks on a stalled reciprocal and
                head-of-line-blocks the LIF ops interleaved behind it."""
                theta = float(2.0 ** t)
                oa = []
                for g in range(G):
                    oa_g = oap.tile([128, QTOK], F16, tag=f"oa{g}",
                                    name=f"oa{g}")
                    oa.append(oa_g)

                def normalize(n, ov):
                    # oa[g][:, n] = ovo * (1/ovs); ovs has beta folded in
                    # (ones32 = 1/beta) so oa = beta_t * out.
                    rs = oap.tile([128, G * 128], F32, tag="rs")
                    ov_sums = ov[:, :].rearrange(
                        "p (g x) -> p g x", x=256)[:, :, 128:256]
                    nc.vector.reciprocal_approx_fast(
                        rs[:, :].rearrange("p (g c) -> p g c", c=128),
                        ov_sums)
                    for g in range(G):
                        nc.vector.tensor_tensor(
                            oa[g][:, n * 128:(n + 1) * 128],
                            ov[:, g * 256:g * 256 + 128],
                            rs[:, g * 128:(g + 1) * 128],
                            mybir.AluOpType.mult)

                pending_norm = None
                for n in range(NLOC):
                    if pending_norm is not None:
                        normalize(*pending_norm)
                        pending_norm = None
                    ov = ovp.tile([128, 512], F32, tag="ov")
                    for g in range(G):
                        ptt = ptp.tile([128, 2048], F16, tag="ptt")
                        for half in range(2):
                            # S^T halves (heads 2*half..2*half+2), 4-way
                            # row-tiled; separate PSUM tiles (bufs=2) so the
                            # next half's matmuls overlap this half's exp.
                            stt = stp.tile([128, 1024], F32, tag="st")
                            for mp in range(NLOC):
                                s = n * TOPK + mp
                                for hh in range(2):
                                    h = 2 * half + hh
                                    nc.tensor.matmul(
                                        stt[:, hh * 512 + mp * 128:
                                            hh * 512 + (mp + 1) * 128],
                                        kv_sel[32 * h:32 * (h + 1),
                                               s * 512 + g * 128:
                                               s * 512 + (g + 1) * 128],
                                        qs[32 * h:32 * (h + 1),
                                           g * QTOK + n * 128:
                                           g * QTOK + (n + 1) * 128],
                                        start=True, stop=True,
                                        tile_position=(32 * h, 0))
                            # NOTE: a DVE Schraudolph-exp variant (writing
                            # ptt via a .bitcast(int16) AP) raced PV reads of
                            # ptt on hardware (dep tracking misses the
                            # bitcast view) — intermittent spurious z-spikes
                            # on fresh-process runs.  All exp stays on ACT.
                            nc.scalar.activation(
                                ptt[:, half * 1024:(half + 1) * 1024],
                                stt[:, :],
                                mybir.ActivationFunctionType.Exp,
                                bias=0.0, scale=TAU_SCALE)
                        # P@V and P@ones: 4-way col-tiled, h innermost
                        for mp in range(NLOC):
                            for h in range(4):
                                hg = g * 4 + h
                                s = n * TOPK + mp
                                nc.tensor.matmul(
                                    ov[32 * h:32 * (h + 1),
                                       g * 256:g * 256 + 128],
                                    kv_sel[:, s * 512 + 256 + hg * HD:
                                           s * 512 + 256 + (hg + 1) * HD],
                                    ptt[:, h * 512 + mp * 128:
                                        h * 512 + (mp + 1) * 128],
                                    start=(mp == 0), stop=(mp == 3),
                                    tile_position=(0, 32 * h))
                            for h in range(4):
                                nc.tensor.matmul(
                                    ov[32 * h:32 * (h + 1),
                                       g * 256 + 128:g * 256 + 256],
                                    ones32[:, :],
                                    ptt[:, h * 512 + mp * 128:
                                        h * 512 + (mp + 1) * 128],
                                    start=(mp == 0), stop=(mp == 3),
                                    tile_position=(0, 32 * h))
                    pending_norm = (n, ov)
                    yield

                normalize(*pending_norm)

                # ---- proj (transposed layout, f16 single pass) + LIF
                for go in range(G):
                    ps = ovp.tile([128, 512], F32, tag="ov")
                    for kc in range(KCH):
                        nc.tensor.matmul(
                            ps[:, :],
                            wp[:, kc, go * 128:(go + 1) * 128],
                            oa[kc][:, :],
                            start=(kc == 0), stop=(kc == 1))
                    zs = oap.tile([128, QTOK], F16, tag="zs")
                    lif_step(uz[:, go * QTOK:(go + 1) * QTOK], ps[:, :],
                             zs[:, :], t == 0, t == T - 1,
                             bqkp[:, go, 2:3], theta)
                    nc.gpsimd.dma_start(zout[t, go, :, :], zs[:, :])
                    yield

            prefetch(0)
            attn_gen = None
            attn_args = None
            for t in range(T):
                inv_beta = float(2.0 ** (1 - t))   # 1/beta_t
                if t + 1 < T:
                    prefetch(t + 1)
                xs = xs_tiles.pop(t)
                bqkp, bvt = b_tiles.pop(t)
                ones32 = biasp.tile([128, HD], F16, tag="ones32")
                nc.vector.memset(ones32[:, :], inv_beta)

                # interleave attention(t-1) units with qkv(t) chunks so each
                # engine's in-order queue alternates between the two streams
                qkv_gen = emit_qkv_gather(t, xs, bqkp, bvt)
                nq = 4 + 4 + 1 + 2       # k chunks + v pairs + gather + q
                na = NLOC + G if attn_gen is not None else 0
                qi = ai = 0
                while qi < nq or ai < na:
                    # pace attention units evenly through the qkv stream
                    if ai < na and (qi >= nq or ai * nq <= qi * na):
                        next(attn_gen, None)
                        ai += 1
                    else:
                        next(qkv_gen, None)
                        qi += 1
                for _ in qkv_gen:
                    pass
                kv_sel, qs = _RESULT.pop(t)
                attn_gen = emit_attention(t, kv_sel, qs, ones32, bqkp)
            for _ in attn_gen:
                pass

    # populate .instr bytes for InstISA subclasses (custom DVE ops); raw
    # Bass skips Bacc's codegen pass and walrus dies with "ISA wrong
    # length" on empty .instr otherwise.
    mybir.codegen_inst_isa_subclasses(nc)
    return _wrap_to_json(nc)


# ------------------------------------------------------------------ driver
_CACHE = {}


def kernel(x, w_qkv, b_qkv, w_proj, b_proj):
    from concourse.bass_utils import run_bass_kernel_spmd

    x = np.asarray(x, dtype=np.float32)
    w_qkv = np.asarray(w_qkv, dtype=np.float32)
    b_qkv = np.asarray(b_qkv, dtype=np.float32)
    w_proj = np.asarray(w_proj, dtype=np.float32)
    b_proj = np.asarray(b_proj, dtype=np.float32)

    idx = _routing_idx(x)
    betas = np.asarray([2.0 ** (t - 1) for t in range(T)], np.float32)

    wq_f, wk_f, wv_f = w_qkv[:, :C], w_qkv[:, C:2 * C], w_qkv[:, 2 * C:]
    bqv, bkv, bvv = b_qkv[:C], b_qkv[C:2 * C], b_qkv[2 * C:]

    shared = dict(
        wq=wq_f.astype(np.float16), wk=wk_f.astype(np.float16),
        wv=wv_f.astype(np.float16), wp=w_proj.astype(np.float16),
        bqkp=np.stack([betas[:, None] * bqv[None, :],
                       betas[:, None] * bkv[None, :],
                       betas[:, None] * b_proj[None, :]],
                      axis=2).astype(np.float32),
        bv=np.broadcast_to(
            (betas[:, None] * bvv[None, :] / 128.0).astype(np.float16)[:, None, :],
            (T, 128, C)).copy(),
    )

    in_maps = []
    for core in range(N_CORES):
        b, half = core // 2, core % 2
        perm = list(range(half * 4, half * 4 + 4)) + \
            [w for w in range(N_WIN) if not (half * 4 <= w < half * 4 + 4)]
        pos_of = {w: i for i, w in enumerate(perm)}
        xb = x[:, b]                                         # [T, 1024, C]
        xperm = np.concatenate([xb[:, w * WIN:(w + 1) * WIN] for w in perm],
                               axis=1)                       # [T, 1024, C]
        xsT = np.swapaxes(xperm, 1, 2)                       # [T, C, 1024]
        xs_arr = np.ascontiguousarray(
            xsT * betas[:, None, None]).astype(np.float16)
        wsel = np.zeros(NSLOT, np.int64)
        for n in range(NLOC):
            for j in range(TOPK):
                wsel[n * TOPK + j] = pos_of[int(idx[b, half * 4 + n, j])]
        gidx = (np.arange(128, dtype=np.int64)[:, None] * N_WIN +
                wsel[None, :]).astype(np.int32)              # [128, 16]
        m = dict(shared)
        m.update(xs=xs_arr, gidx=gidx)
        in_maps.append(m)

    key = "prog"
    if key not in _CACHE:
        _CACHE[key] = _build_program()
    nc = _CACHE[key]

    trace = bool(int(os.environ.get("BK_TRACE", "0")))
    res = run_bass_kernel_spmd(
        nc, in_maps, core_ids=list(range(N_CORES)), trace=trace)
    if trace and res.exec_time_ns:
        print(f"HW exec time: {res.exec_time_ns} ns")

    out = np.zeros((T, B, L, C), np.float32)
    for core in range(N_CORES):
        b, half = core // 2, core % 2
        z = res.results[core]["z"]                           # [T, G, 128, 512]
        zt = np.concatenate([z[:, 0], z[:, 1]], axis=1)      # [T, 256, 512]
        out[:, b, half * QTOK:(half + 1) * QTOK, :] = \
            np.swapaxes(zt, 1, 2).astype(np.float32)
    return out
